# revision 1
# baseline (speedup 1.0000x reference)
"""Trainium2 Bass kernel: full (non-causal) softmax attention.

Input:  query/key/value [1, 4096, 16, 128] f32 (B, S, H, D).
Output: [1, 4096, 16, 128] f32 = softmax(Q K^T / sqrt(D)) V per head.

Sharding: 16 heads over 8 cores -> 2 heads per core, no collectives.
Host pre-transposes Q,K per head to [D, S]; the device returns the
UN-normalized attention output transposed [D, S] plus the softmax
denominator row [S]; the host does the final divide (cheap numpy).

Device pipeline, per head, per query-chunk QC (1024 queries):
  for kt in 32 key-chunks (128 keys each):
    ST[kt] = scores^T chunk: psum[128k, QCq]  (two N=512 fp32r matmuls,
             stationary KT chunk; moving operand = QT columns)
    PT[kt] = exp(ST / sqrt(128))              (ACT, psum->sbuf, fp32r)
    OUT   += V_kt^T @ PT[kt]                  (accumulating fp32r matmuls)
    den   += colsum(PT[kt])   split across PE (ones-vector matmuls),
             DVE and GPSIMD (tensor adds) to balance engine load
ACT (exp) is the throughput floor (~280us); everything else is tuned
to stay below it.
"""

import os
import sys
from contextlib import ExitStack

import numpy as np

sys.path.insert(0, "/opt/trn_rl_repo")

import concourse.bacc as bacc
import concourse.bass as bass
import concourse.tile as tile
from concourse import mybir
from concourse.bass_utils import run_bass_kernel_spmd

N_CORES = 8
S = 4096
H = 16
D = 128
HEADS_PER_CORE = H // N_CORES  # 2
KT_CHUNK = 128                  # keys per score tile (psum partition dim)
QC = 1024                       # queries per super-chunk (ACT tile free dim)
NMM = 512                       # moving free dim per matmul (psum bank, fp32 max)
SCALE = float(D) ** -0.5

F32 = mybir.dt.float32
F32R = mybir.dt.float32r

# per-32-chunk denominator-reduction role assignment (tuned from trace):
# 'P' = PE ones-matmul, 'V' = DVE tensor add, 'G' = GPSIMD tensor add
N_PE_DEN = 5
N_DVE_DEN = 27


def _den_roles(n_kt):
    roles = []
    for kt in range(n_kt):
        r = kt % 32
        if r < N_PE_DEN:
            roles.append("P")
        elif r < N_PE_DEN + N_DVE_DEN:
            roles.append("V")
        else:
            roles.append("G")
    # interleave so same-engine work is spread across the loop
    order = sorted(range(n_kt), key=lambda i: (i * 13) % n_kt)
    out = [None] * n_kt
    for slot, role in zip(order, roles):
        out[slot] = role
    return out


def build_program(s=S, heads=HEADS_PER_CORE):
    nc = bacc.Bacc("TRN2", target_bir_lowering=False, debug=False,
                   num_devices=N_CORES)

    n_kt = s // KT_CHUNK
    n_qc = s // QC
    roles = _den_roles(n_kt)

    qt_d = nc.dram_tensor("qt", [heads, D, s], F32, kind="ExternalInput")
    kt_d = nc.dram_tensor("kt", [heads, D, s], F32, kind="ExternalInput")
    v_d = nc.dram_tensor("v", [heads, s, D], F32, kind="ExternalInput")
    out_d = nc.dram_tensor("out", [heads, D, s], F32, kind="ExternalOutput")
    den_d = nc.dram_tensor("den", [heads, s], F32, kind="ExternalOutput")

    with tile.TileContext(nc) as tc, ExitStack() as ctx:
        consts = ctx.enter_context(tc.tile_pool(name="consts", bufs=1))
        qkv_pool = ctx.enter_context(tc.tile_pool(name="qkv", bufs=2))
        pt_pool = ctx.enter_context(tc.tile_pool(name="pt", bufs=10))
        acc_pool = ctx.enter_context(tc.tile_pool(name="acc", bufs=2))
        osb_pool = ctx.enter_context(tc.tile_pool(name="osb", bufs=3))
        densb_pool = ctx.enter_context(tc.tile_pool(name="densb", bufs=2))
        st_pool = ctx.enter_context(
            tc.tile_pool(name="st", bufs=2, space="PSUM"))
        outp_pool = ctx.enter_context(
            tc.tile_pool(name="outp", bufs=2, space="PSUM"))
        denp_pool = ctx.enter_context(
            tc.tile_pool(name="denp", bufs=1, space="PSUM"))

        ones_f = consts.tile([128, 1], F32, tag="ones_f")
        nc.vector.memset(ones_f[:], 1.0)
        ones_col = consts.tile([128, 1], F32R, tag="ones_col")
        nc.scalar.copy(ones_col[:], ones_f[:])

        # Per-head on-chip tensors (double-buffered across heads).
        # float32r tiles are bit-identical to f32; typing the producers
        # f32r keeps the BIR verifier happy for fp32r matmul consumers.
        def load_head(h):
            qt_sb = qkv_pool.tile([D, s], F32R, tag="qt")
            nc.sync.dma_start(out=qt_sb[:], in_=qt_d[h].bitcast(F32R))
            kt_sb = qkv_pool.tile([D, s], F32R, tag="kt")
            nc.sync.dma_start(out=kt_sb[:], in_=kt_d[h].bitcast(F32R))
            v_sb = qkv_pool.tile([128, n_kt, D], F32R, tag="v")
            nc.sync.dma_start(
                out=v_sb[:],
                in_=v_d[h].rearrange("(c p) d -> p c d", p=128).bitcast(F32R))
            return qt_sb, kt_sb, v_sb

        heads_sb = [load_head(0)]

        # Deferred epilogue work, interleaved into the next chunk's matmul
        # stream so the PE pipeline never waits on DVE.
        pending = []

        for h in range(heads):
            qt_sb, kt_sb, v_sb = heads_sb[h]
            if h + 1 < heads:
                heads_sb.append(load_head(h + 1))
            for qc in range(n_qc):
                q0 = qc * QC
                out_ps = [outp_pool.tile([D, NMM], F32, tag="outp",
                                         name=f"out_ps{j}")
                          for j in range(QC // NMM)]
                den_ps = denp_pool.tile([1, QC], F32, tag="denp")
                accs = {"V": [], "G": []}
                den_started = [False] * (QC // NMM)
                for kt in range(n_kt):
                    k0 = kt * KT_CHUNK
                    st = st_pool.tile([128, QC], F32, tag="st")
                    lhs_k = kt_sb[:, k0:k0 + KT_CHUNK]
                    for j in range(QC // NMM):
                        nc.tensor.matmul(
                            st[:, j * NMM:(j + 1) * NMM],
                            lhs_k,
                            qt_sb[:, q0 + j * NMM:q0 + (j + 1) * NMM],
                            start=True, stop=True)
                    pt = pt_pool.tile([128, QC], F32R, tag="pt")
                    nc.scalar.activation(
                        pt[:], st[:], mybir.ActivationFunctionType.Exp,
                        scale=SCALE)
                    lhs_v = v_sb[:, kt, :]
                    for j in range(QC // NMM):
                        nc.tensor.matmul(
                            out_ps[j][:],
                            lhs_v,
                            pt[:, j * NMM:(j + 1) * NMM],
                            start=(kt == 0), stop=(kt == n_kt - 1))
                    # softmax denominator partial reduction.
                    role = roles[kt]
                    if role == "P":
                        for j in range(QC // NMM):
                            nc.tensor.matmul(
                                den_ps[:, j * NMM:(j + 1) * NMM],
                                ones_col[:],
                                pt[:, j * NMM:(j + 1) * NMM],
                                start=(not den_started[j]), stop=False,
                                skip_group_check=True)
                            den_started[j] = True
                    else:
                        eng = nc.vector if role == "V" else nc.gpsimd
                        ptf = pt[:].bitcast(F32)
                        lst = accs[role]
                        if not lst:
                            a = acc_pool.tile([128, QC], F32, tag="acc" + role,
                                              name="acc" + role)
                            eng.tensor_copy(a[:], ptf)
                            lst.append(a)
                        else:
                            b = acc_pool.tile([128, QC], F32,
                                              tag="acc" + role + "b",
                                              name="acc" + role + "b")
                            eng.tensor_add(b[:], lst[-1][:], ptf)
                            lst.append(b)
                    if pending:
                        pending.pop(0)()

                def finish(out_ps=out_ps, den_ps=den_ps, accs=accs, h=h,
                           q0=q0, den_started=den_started):
                    folds = []
                    if accs["V"] and accs["G"]:
                        accm = acc_pool.tile([128, QC], F32, tag="accm")
                        nc.vector.tensor_add(accm[:], accs["V"][-1][:],
                                             accs["G"][-1][:])
                        folds.append(accm)
                    elif accs["V"] or accs["G"]:
                        folds.append((accs["V"] or accs["G"])[-1])
                    den_sb = densb_pool.tile([1, QC], F32, tag="den_sb")
                    out_sb = osb_pool.tile([D, QC], F32, tag="out_sb")

                    def s1():
                        # fold the DVE/GPSIMD accumulators into the psum
                        # denominator row (plain fp32 matmul: f32 producer).
                        started = list(den_started)
                        for fi, acc in enumerate(folds):
                            last = fi == len(folds) - 1
                            for j in range(QC // NMM):
                                nc.tensor.matmul(
                                    den_ps[:, j * NMM:(j + 1) * NMM],
                                    ones_f[:],
                                    acc[:, j * NMM:(j + 1) * NMM],
                                    start=(not started[j]), stop=last,
                                    skip_group_check=True)
                                started[j] = True

                    def s2():
                        nc.vector.tensor_copy(den_sb[:], den_ps[:])
                        nc.sync.dma_start(
                            out=den_d[h:h + 1, q0:q0 + QC], in_=den_sb[:])
                        for j in range(QC // NMM):
                            nc.vector.tensor_copy(
                                out_sb[:, j * NMM:(j + 1) * NMM],
                                out_ps[j][:])
                        nc.sync.dma_start(
                            out=out_d[h][:, q0:q0 + QC], in_=out_sb[:])

                    return [s1, s2]

                pending.extend(finish())
        while pending:
            pending.pop(0)()

    nc.compile()
    return nc


def _install_ntff_hook():
    """Provide antenv.axon_hooks (absent in this image) so that
    run_bass_kernel_spmd(trace=True) can capture NTFF profiles via the
    axon .so — mirrors trn_agent_boot.trn_boot._ntff_profile_via_ctypes."""
    try:
        from antenv.axon_hooks import get_axon_ntff_profile_hook  # noqa: F401
        return
    except ImportError:
        pass
    import contextlib
    import ctypes
    import types

    so_path = "/opt/axon/libaxon_pjrt.so"
    lib = ctypes.CDLL(so_path)
    if not hasattr(lib, "axon_start_nrt_profile"):
        return
    lib.axon_start_nrt_profile.argtypes = [
        ctypes.POINTER(ctypes.c_int64), ctypes.c_size_t]
    lib.axon_start_nrt_profile.restype = ctypes.c_int64
    lib.axon_stop_nrt_profile.argtypes = [ctypes.c_char_p]
    lib.axon_stop_nrt_profile.restype = ctypes.c_int64

    @contextlib.contextmanager
    def _hook(output_dir, device_ids):
        import jax
        jax.devices()
        if device_ids:
            ids = (ctypes.c_int64 * len(device_ids))(*device_ids)
            rc = lib.axon_start_nrt_profile(ids, len(device_ids))
        else:
            rc = lib.axon_start_nrt_profile(None, 0)
        if rc != 0:
            raise RuntimeError(f"axon_start_nrt_profile rc={rc}")
        try:
            yield
        finally:
            n = lib.axon_stop_nrt_profile(str(output_dir).encode())
            print(f"ntff profile: {n} file(s) written to {output_dir}")

    mod = types.ModuleType("antenv.axon_hooks")
    mod.get_axon_ntff_profile_hook = lambda: _hook
    mod.set_axon_ntff_profile_hook = lambda h: None
    import antenv
    sys.modules["antenv.axon_hooks"] = mod
    antenv.axon_hooks = mod


_CACHE = {}


def _get_program():
    key = "main"
    if key not in _CACHE:
        _CACHE[key] = build_program()
    return _CACHE[key]


def kernel(query, key, value, trace=False, **trace_kwargs):
    assert query.shape == (1, S, H, D)
    nc = _get_program()

    q = np.asarray(query, dtype=np.float32)[0]   # [S, H, D]
    k = np.asarray(key, dtype=np.float32)[0]
    v = np.asarray(value, dtype=np.float32)[0]

    in_maps = []
    for c in range(N_CORES):
        hs = slice(c * HEADS_PER_CORE, (c + 1) * HEADS_PER_CORE)
        # [S, h, D] -> [h, D, S]
        qt = np.ascontiguousarray(q[:, hs, :].transpose(1, 2, 0))
        kt = np.ascontiguousarray(k[:, hs, :].transpose(1, 2, 0))
        vv = np.ascontiguousarray(v[:, hs, :].transpose(1, 0, 2))
        in_maps.append({"qt": qt, "kt": kt, "v": vv})

    if trace:
        _install_ntff_hook()
    res = run_bass_kernel_spmd(nc, in_maps, core_ids=list(range(N_CORES)),
                               trace=trace, **trace_kwargs)

    out = np.empty((1, S, H, D), dtype=np.float32)
    for c in range(N_CORES):
        o = res.results[c]["out"]    # [h, D, S] unnormalized
        den = res.results[c]["den"]  # [h, S]
        for i in range(HEADS_PER_CORE):
            out[0, :, c * HEADS_PER_CORE + i, :] = (o[i] / den[i][None, :]).T
    if trace:
        kernel.last_results = res
    return out

